# revision 11
# baseline (speedup 1.0000x reference)
"""Trainium2 Bass kernel for CurlVectorField.

curl(psi) where psi = W3 tanh(W2 tanh(W1 x + b1) + b2) + b3, x in R^3,
N = 524288 points. Data-parallel over 8 NeuronCores.

Math (per point): S = tanh(W1x+b1), D1 = 1-S^2, D2 = 1-tanh^2(W2 S+b2),
  B_c = antisym combos of W3-rows x (W2 * W1-cols)   (host-folded)
  curl_c = sum_h D2[h] * (B_c @ D1)[h]

Layout: TWO points per column ("2-pack") - H=64 so [pt_even; pt_odd]
features fill all 128 partitions. Per iteration (1024 points, 512 cols):
  psum1 = blockdiag(W1T,W1T).T @ xt6        f32r    [PE 213ns]
  S     = tanh(psum1 + b1d)                 ACT -> bf16
  S2    = S*S                               Pool tt (SBUF only - gpsimd
  D1    = 1 - S2                            Pool ts  cannot touch PSUM)
  psum2 = blockdiag(W2T,W2T).T @ S          bf16    [PE]
  T2    = tanh(psum2 + b2d)                 ACT -> bf16
  T2sq  = Square(T2)                        ACT
  D2    = 1 - T2sq                          DVE ts (4x)
  psumY[:,c,:] = blockdiag(Bc.T) @ D1       bf16 x3 [PE]
  V     = psumY * D2 (bcast)                DVE triple op (only DVE/ACT
                                            read PSUM; ACT cannot mult)
  psum5: 3 gsel reduce matmuls (bf16 stationary [128,6], tile_position
      row-strips 0/32/64) accumulate 3 iterations into one bank.
  every 3 iters: ACT copy psum5 -> SBUF f32, DMA three 6-row strips out.

No gcst pass / no cst constants needed: psumY uses D1 directly (the
baseline's -B/S^2/cst fold traded a cheap elementwise op for a PE pass).
Per-iter engine busy: PE 1704, ACT 1836, DVE 1919, Pool 1917 ns.
Host packs x as (6, NSH/2) [even xyz; odd xyz] and unpacks yt6 (6, NSH/2).
"""

import os
import sys
from contextlib import ExitStack

import numpy as np

sys.path.insert(0, "/opt/trn_rl_repo")

import dataclasses

import concourse.bass as bass
import concourse.bacc as bacc
import concourse.tile as tile
from concourse import mybir
from concourse.bass_utils import run_bass_kernel_spmd

N_CORES = 8
NPTS = 524288
NSH = NPTS // N_CORES          # 65536 points per core
NSH2 = NSH // 2                # 32768 columns per core
H = 64
TILE_N = 512
NT = NSH2 // TILE_N            # 64 iterations per core
XL = 342                       # V01 column split: [0:XL] on DVE, [XL:] Pool

F32 = mybir.dt.float32
F32R = mybir.dt.float32r
BF16 = mybir.dt.bfloat16


def _bcast(ap, dim, count):
    """Stride-0 broadcast of `ap` along axis `dim` (which must have size 1)."""
    newap = [list(p) for p in ap.ap]
    assert newap[dim][1] == 1
    newap[dim] = [0, count]
    return dataclasses.replace(ap, ap=type(ap.ap)(newap))


def _build_program():
    nc = bacc.Bacc(
        "TRN2",
        target_bir_lowering=False,
        debug=False,
        num_devices=N_CORES,
    )

    xt6 = nc.dram_tensor("xt6", [6, NSH2], F32R, kind="ExternalInput").ap()
    w1bd = nc.dram_tensor("w1bd", [6, 128], F32R, kind="ExternalInput").ap()
    b1d = nc.dram_tensor("b1d", [128, 1], F32, kind="ExternalInput").ap()
    w2bd = nc.dram_tensor("w2bd", [128, 128], BF16, kind="ExternalInput").ap()
    b2d = nc.dram_tensor("b2d", [128, 1], F32, kind="ExternalInput").ap()
    bBd = nc.dram_tensor("bBd", [128, 3, 128], BF16, kind="ExternalInput").ap()
    gsel = nc.dram_tensor("gsel", [128, 3, 6], BF16, kind="ExternalInput").ap()
    yt6 = nc.dram_tensor("yt6", [6, NSH2], F32, kind="ExternalOutput").ap()

    with tile.TileContext(nc) as tc, ExitStack() as ctx:
        consts = ctx.enter_context(tc.tile_pool(name="consts", bufs=1))
        xin = ctx.enter_context(tc.tile_pool(name="xin", bufs=3))
        sb = ctx.enter_context(tc.tile_pool(name="sb", bufs=3))
        outp = ctx.enter_context(tc.tile_pool(name="outp", bufs=2))
        pp1 = ctx.enter_context(tc.tile_pool(name="pp1", bufs=2, space="PSUM"))
        pp2 = ctx.enter_context(tc.tile_pool(name="pp2", bufs=2, space="PSUM"))
        ppy = ctx.enter_context(tc.tile_pool(name="ppy", bufs=1, space="PSUM"))
        pp5 = ctx.enter_context(tc.tile_pool(name="pp5", bufs=1, space="PSUM"))

        w1bd_s = consts.tile([6, 128], F32R)
        b1d_s = consts.tile([128, 1], F32)
        w2bd_s = consts.tile([128, 128], BF16)
        b2d_s = consts.tile([128, 1], F32)
        bBd_s = consts.tile([128, 3, 128], BF16)
        gsel_s = consts.tile([128, 3, 6], BF16)
        for dst, src in (
            (w1bd_s, w1bd), (b1d_s, b1d), (w2bd_s, w2bd), (b2d_s, b2d),
            (bBd_s, bBd), (gsel_s, gsel),
        ):
            nc.sync.dma_start(out=dst, in_=src)

        psum5q = None
        pend = []
        for t in range(NT):
            sl = slice(t * TILE_N, (t + 1) * TILE_N)

            xt_t = xin.tile([6, TILE_N], F32R)
            nc.sync.dma_start(out=xt_t, in_=xt6[:, sl])

            psum1 = pp1.tile([128, TILE_N], F32, tag="psum1")
            nc.tensor.matmul(psum1, w1bd_s[:, :], xt_t[:, :],
                             start=True, stop=True)

            S = sb.tile([128, TILE_N], BF16, tag="S")
            nc.scalar.activation(S[:, :], psum1[:, :],
                                 mybir.ActivationFunctionType.Tanh,
                                 bias=b1d_s[:, :])
            S2 = sb.tile([128, TILE_N], BF16, tag="S2")
            nc.gpsimd.tensor_mul(S2[:, :], S[:, :], S[:, :])
            D1 = sb.tile([128, TILE_N], BF16, tag="D1")
            nc.gpsimd.tensor_scalar(D1[:, :], S2[:, :], -1.0, 1.0,
                                    mybir.AluOpType.mult,
                                    mybir.AluOpType.add)

            psum2 = pp2.tile([128, TILE_N], F32, tag="psum2")
            nc.tensor.matmul(psum2, w2bd_s[:, :], S[:, :],
                             start=True, stop=True)

            T2 = sb.tile([128, TILE_N], BF16, tag="T2")
            nc.scalar.activation(T2[:, :], psum2[:, :],
                                 mybir.ActivationFunctionType.Tanh,
                                 bias=b2d_s[:, :])
            T2sq = sb.tile([128, TILE_N], BF16, tag="T2sq")
            nc.scalar.activation(T2sq[:, :], T2[:, :],
                                 mybir.ActivationFunctionType.Square)
            D2 = sb.tile([128, TILE_N], BF16, tag="D2")
            nc.vector.tensor_scalar(D2[:, :], T2sq[:, :], -1.0, 1.0,
                                    mybir.AluOpType.mult,
                                    mybir.AluOpType.add)

            psumY = ppy.tile([128, 3, TILE_N], F32, tag="psumY")
            for c in range(3):
                nc.tensor.matmul(psumY[:, c, :], bBd_s[:, c, :], D1[:, :],
                                 start=True, stop=True)

            # V = psumY * D2 (broadcast over c) - one DVE triple op
            V = sb.tile([128, 3, TILE_N], BF16, tag="V")
            D2b = _bcast(D2[:, None, :], 1, 3)
            nc.vector.tensor_mul(V[:, :, :], psumY[:, :, :], D2b)

            # reduce: 3 gsel matmuls into a 6-row strip at partition 32*(t%3)
            # (matmul base partition must be 0/32/64, so 3 strips per bank)
            r = t % 3
            if r == 0:
                psum5q = pp5.tile([128, TILE_N], F32, tag="psum5q")
            r0 = 32 * r
            last = (r == 2) or (t == NT - 1)
            for c in range(3):
                nc.tensor.matmul(psum5q[r0:r0 + 6, :], gsel_s[:, c, :],
                                 V[:, c, :],
                                 start=(c == 0), stop=(last and c == 2),
                                 skip_group_check=True)
            pend.append((r0, sl))

            if last:
                yq = outp.tile([128, TILE_N], F32, tag="yq")
                nc.scalar.copy(yq[:, :], psum5q[:, :])
                for (rb, ssl) in pend:
                    nc.sync.dma_start(out=yt6[:, ssl],
                                      in_=yq[rb:rb + 6, :])
                pend = []

    nc.compile()
    return nc


_NC_CACHE = None


def _get_program():
    global _NC_CACHE
    if _NC_CACHE is None:
        _NC_CACHE = _build_program()
    return _NC_CACHE


def _host_weights(W1, b1, W2, b2, W3):
    import ml_dtypes
    W1 = np.asarray(W1, np.float32)
    W2 = np.asarray(W2, np.float32)
    W3 = np.asarray(W3, np.float32)
    b1 = np.asarray(b1, np.float32)
    b2 = np.asarray(b2, np.float32)
    M = np.einsum("hk,kj->jhk", W2, W1)          # M_j = W2 * W1[:,j]
    B = np.stack([
        W3[2][:, None] * M[1] - W3[1][:, None] * M[2],
        W3[0][:, None] * M[2] - W3[2][:, None] * M[0],
        W3[1][:, None] * M[0] - W3[0][:, None] * M[1],
    ]).astype(np.float32)                         # (3, H, H)

    Z = np.zeros((64, 64), np.float32)
    bd = lambda A: np.block([[A, Z], [Z, A]]).astype(np.float32)

    w1bd = np.zeros((6, 128), np.float32)
    w1bd[0:3, 0:64] = W1.T
    w1bd[3:6, 64:128] = W1.T

    # reduce selectors: pass c sums rows 0:64 (even pt) into out row c and
    # rows 64:128 (odd pt) into out row 3+c
    gsel = np.zeros((3, 128, 6), np.float32)
    for c in range(3):
        gsel[c, 0:64, c] = 1.0
        gsel[c, 64:128, 3 + c] = 1.0

    bf = ml_dtypes.bfloat16
    c_ = np.ascontiguousarray
    return {
        "w1bd": c_(w1bd),
        "b1d": c_(np.concatenate([b1, b1])[:, None]),
        "w2bd": c_(bd(W2.T).astype(bf)),
        "b2d": c_(np.concatenate([b2, b2])[:, None]),
        "bBd": c_(np.stack([bd(B[c].T) for c in range(3)], axis=1).astype(bf)),
        "gsel": c_(gsel.transpose(1, 0, 2).astype(bf)),
    }


def kernel(x, W1, b1, W2, b2, W3, b3, _want_trace=False):
    x = np.asarray(x, np.float32)
    wts = _host_weights(W1, b1, W2, b2, W3)

    in_maps = []
    for ci in range(N_CORES):
        xs = x[ci * NSH:(ci + 1) * NSH]                       # (NSH, 3)
        xt6 = np.ascontiguousarray(
            xs.reshape(NSH2, 2, 3).transpose(1, 2, 0).reshape(6, NSH2))
        m = {"xt6": xt6}
        m.update(wts)
        in_maps.append(m)

    nc = _get_program()
    res = None
    for attempt in range(3):
        try:
            res = run_bass_kernel_spmd(nc, in_maps, list(range(N_CORES)),
                                       trace=_want_trace)
            break
        except Exception as e:
            # Axon-tunneled NeuronCores occasionally report a transient
            # NRT_EXEC_UNIT_UNRECOVERABLE; a retry on the same devices
            # consistently succeeds.
            if attempt == 2 or "UNRECOVERABLE" not in str(e).upper():
                raise
            import time
            time.sleep(10)
    outs = []
    for ci in range(N_CORES):
        yt6 = res.results[ci]["yt6"]                          # (6, NSH2)
        y = yt6.reshape(2, 3, NSH2).transpose(2, 0, 1).reshape(NSH, 3)
        outs.append(y)
    out = np.ascontiguousarray(np.concatenate(outs, axis=0)).astype(np.float32)
    if _want_trace:
        return out, res
    return out


# revision 14
# speedup vs baseline: 1.7087x; 1.7087x over previous
"""Trainium2 Bass kernel for CurlVectorField.

curl(psi) where psi = W3 tanh(W2 tanh(W1 x + b1) + b2) + b3, x in R^3,
N = 524288 points. Data-parallel over 8 NeuronCores.

Math (per point): S = tanh(W1x+b1), D1 = 1-S^2, D2 = 1-tanh^2(W2 S+b2),
  B_c = antisym combos of W3-rows x (W2 * W1-cols)   (host-folded)
  curl_c = sum_h D2[h] * (B_c @ D1)[h]

Layout: TWO points per column ("2-pack") - H=64 so [pt_even; pt_odd]
features fill all 128 partitions. Per iteration (1024 points, 512 cols):
  psum1 = blockdiag(W1T,W1T).T @ xt6        f32r    [PE 213ns]
  S     = tanh(psum1 + b1d)                 ACT -> bf16
  S2    = S*S                               Pool tt (SBUF only - gpsimd
  D1    = 1 - S2                            Pool ts  cannot touch PSUM)
  psum2 = blockdiag(W2T,W2T).T @ S          bf16    [PE]
  T2    = tanh(psum2 + b2d)                 ACT -> bf16
  T2sq  = Square(T2)                        ACT
  D2    = 1 - T2sq                          DVE ts (4x)
  psumY[:,c,:] = blockdiag(Bc.T) @ D1       bf16 x3 [PE]
  V     = psumY * D2 (bcast)                DVE triple op (only DVE/ACT
                                            read PSUM; ACT cannot mult)
  psum5: 3 gsel reduce matmuls (bf16 stationary [128,6], tile_position
      row-strips 0/32/64) accumulate 3 iterations into one bank.
  every 3 iters: ACT copy psum5 -> SBUF f32, DMA three 6-row strips out.

No gcst pass / no cst constants needed: psumY uses D1 directly (the
baseline's -B/S^2/cst fold traded a cheap elementwise op for a PE pass).
Per-iter engine busy: PE 1704, ACT 1836, DVE 1919, Pool 1917 ns.
Host packs x as (6, NSH/2) [even xyz; odd xyz] and unpacks yt6 (6, NSH/2).
"""

import os
import sys
from contextlib import ExitStack

import numpy as np

sys.path.insert(0, "/opt/trn_rl_repo")

import dataclasses

import concourse.bass as bass
import concourse.bacc as bacc
import concourse.tile as tile
from concourse import mybir
from concourse.bass_utils import run_bass_kernel_spmd

N_CORES = 8
NPTS = 524288
NSH = NPTS // N_CORES          # 65536 points per core
NSH2 = NSH // 2                # 32768 columns per core
H = 64
TILE_N = 512
NT = NSH2 // TILE_N            # 64 iterations per core
XL = 342                       # V01 column split: [0:XL] on DVE, [XL:] Pool

F32 = mybir.dt.float32
F32R = mybir.dt.float32r
BF16 = mybir.dt.bfloat16


def _bcast(ap, dim, count):
    """Stride-0 broadcast of `ap` along axis `dim` (which must have size 1)."""
    newap = [list(p) for p in ap.ap]
    assert newap[dim][1] == 1
    newap[dim] = [0, count]
    return dataclasses.replace(ap, ap=type(ap.ap)(newap))


def _build_program():
    nc = bacc.Bacc(
        "TRN2",
        target_bir_lowering=False,
        debug=False,
        num_devices=N_CORES,
    )

    xt6 = nc.dram_tensor("xt6", [6, NSH2], F32R, kind="ExternalInput").ap()
    w1bd = nc.dram_tensor("w1bd", [6, 128], F32R, kind="ExternalInput").ap()
    b1d = nc.dram_tensor("b1d", [128, 1], F32, kind="ExternalInput").ap()
    w2bd = nc.dram_tensor("w2bd", [128, 128], BF16, kind="ExternalInput").ap()
    b2d = nc.dram_tensor("b2d", [128, 1], F32, kind="ExternalInput").ap()
    bBd = nc.dram_tensor("bBd", [128, 3, 128], BF16, kind="ExternalInput").ap()
    gsel = nc.dram_tensor("gsel", [128, 3, 6], BF16, kind="ExternalInput").ap()
    yt6 = nc.dram_tensor("yt6", [6, NSH2], F32, kind="ExternalOutput").ap()

    with tile.TileContext(nc) as tc, ExitStack() as ctx:
        consts = ctx.enter_context(tc.tile_pool(name="consts", bufs=1))
        xin = ctx.enter_context(tc.tile_pool(name="xin", bufs=3))
        sb = ctx.enter_context(tc.tile_pool(name="sb", bufs=3))
        outp = ctx.enter_context(tc.tile_pool(name="outp", bufs=2))
        # PSUM budget (8 banks): psumY01 pair double-buffered (4) breaks the
        # V->PE->V serialization cycle; psumY2 single (1) is evacuated first
        # by its own DVE op so its bank frees early; psum1/psum2/psum5 1 each.
        pp1 = ctx.enter_context(tc.tile_pool(name="pp1", bufs=1, space="PSUM"))
        pp2 = ctx.enter_context(tc.tile_pool(name="pp2", bufs=1, space="PSUM"))
        ppy = ctx.enter_context(tc.tile_pool(name="ppy", bufs=2, space="PSUM"))
        ppy2 = ctx.enter_context(tc.tile_pool(name="ppy2", bufs=1, space="PSUM"))
        pp5 = ctx.enter_context(tc.tile_pool(name="pp5", bufs=1, space="PSUM"))

        w1bd_s = consts.tile([6, 128], F32R)
        b1d_s = consts.tile([128, 1], F32)
        w2bd_s = consts.tile([128, 128], BF16)
        b2d_s = consts.tile([128, 1], F32)
        bBd_s = consts.tile([128, 3, 128], BF16)
        gsel_s = consts.tile([128, 3, 6], BF16)
        for dst, src in (
            (w1bd_s, w1bd), (b1d_s, b1d), (w2bd_s, w2bd), (b2d_s, b2d),
            (bBd_s, bBd), (gsel_s, gsel),
        ):
            nc.sync.dma_start(out=dst, in_=src)

        psum5q = None
        pend = []
        for t in range(NT):
            sl = slice(t * TILE_N, (t + 1) * TILE_N)

            xt_t = xin.tile([6, TILE_N], F32R)
            nc.sync.dma_start(out=xt_t, in_=xt6[:, sl])

            psum1 = pp1.tile([128, TILE_N], F32, tag="psum1")
            nc.tensor.matmul(psum1, w1bd_s[:, :], xt_t[:, :],
                             start=True, stop=True)

            S = sb.tile([128, TILE_N], BF16, tag="S")
            nc.scalar.activation(S[:, :], psum1[:, :],
                                 mybir.ActivationFunctionType.Tanh,
                                 bias=b1d_s[:, :])
            S2 = sb.tile([128, TILE_N], BF16, tag="S2")
            nc.gpsimd.tensor_mul(S2[:, :], S[:, :], S[:, :])
            D1 = sb.tile([128, TILE_N], BF16, tag="D1")
            nc.gpsimd.tensor_scalar(D1[:, :], S2[:, :], -1.0, 1.0,
                                    mybir.AluOpType.mult,
                                    mybir.AluOpType.add)

            psum2 = pp2.tile([128, TILE_N], F32, tag="psum2")
            nc.tensor.matmul(psum2, w2bd_s[:, :], S[:, :],
                             start=True, stop=True)

            T2 = sb.tile([128, TILE_N], BF16, tag="T2")
            nc.scalar.activation(T2[:, :], psum2[:, :],
                                 mybir.ActivationFunctionType.Tanh,
                                 bias=b2d_s[:, :])
            T2sq = sb.tile([128, TILE_N], BF16, tag="T2sq")
            nc.scalar.activation(T2sq[:, :], T2[:, :],
                                 mybir.ActivationFunctionType.Square)
            D2 = sb.tile([128, TILE_N], BF16, tag="D2")
            nc.vector.tensor_scalar(D2[:, :], T2sq[:, :], -1.0, 1.0,
                                    mybir.AluOpType.mult,
                                    mybir.AluOpType.add)

            psumY2 = ppy2.tile([128, TILE_N], F32, tag="psumY2")
            nc.tensor.matmul(psumY2[:, :], bBd_s[:, 2, :], D1[:, :],
                             start=True, stop=True)
            psumY = ppy.tile([128, 2, TILE_N], F32, tag="psumY")
            for c in range(2):
                nc.tensor.matmul(psumY[:, c, :], bBd_s[:, c, :], D1[:, :],
                                 start=True, stop=True)

            # V = psumY * D2 (broadcast over c); V2 first so the single
            # psumY2 bank frees early for iteration t+1
            V2 = sb.tile([128, TILE_N], BF16, tag="V2")
            nc.vector.tensor_mul(V2[:, :], psumY2[:, :], D2[:, :])
            V = sb.tile([128, 2, TILE_N], BF16, tag="V")
            D2b = _bcast(D2[:, None, :], 1, 2)
            nc.vector.tensor_mul(V[:, :, :], psumY[:, :, :], D2b)

            # reduce: 3 gsel matmuls into a 6-row strip at partition 32*(t%3)
            # (matmul base partition must be 0/32/64, so 3 strips per bank)
            r = t % 3
            if r == 0:
                psum5q = pp5.tile([128, TILE_N], F32, tag="psum5q")
            r0 = 32 * r
            last = (r == 2) or (t == NT - 1)
            for c in range(3):
                mov = V[:, c, :] if c < 2 else V2[:, :]
                nc.tensor.matmul(psum5q[r0:r0 + 6, :], gsel_s[:, c, :], mov,
                                 start=(c == 0), stop=(last and c == 2),
                                 skip_group_check=True)
            pend.append((r0, sl))

            if last:
                yq = outp.tile([128, TILE_N], F32, tag="yq")
                nc.scalar.copy(yq[:, :], psum5q[:, :])
                for (rb, ssl) in pend:
                    nc.sync.dma_start(out=yt6[:, ssl],
                                      in_=yq[rb:rb + 6, :])
                pend = []

    nc.compile()
    return nc


_NC_CACHE = None


def _get_program():
    global _NC_CACHE
    if _NC_CACHE is None:
        _NC_CACHE = _build_program()
    return _NC_CACHE


def _host_weights(W1, b1, W2, b2, W3):
    import ml_dtypes
    W1 = np.asarray(W1, np.float32)
    W2 = np.asarray(W2, np.float32)
    W3 = np.asarray(W3, np.float32)
    b1 = np.asarray(b1, np.float32)
    b2 = np.asarray(b2, np.float32)
    M = np.einsum("hk,kj->jhk", W2, W1)          # M_j = W2 * W1[:,j]
    B = np.stack([
        W3[2][:, None] * M[1] - W3[1][:, None] * M[2],
        W3[0][:, None] * M[2] - W3[2][:, None] * M[0],
        W3[1][:, None] * M[0] - W3[0][:, None] * M[1],
    ]).astype(np.float32)                         # (3, H, H)

    Z = np.zeros((64, 64), np.float32)
    bd = lambda A: np.block([[A, Z], [Z, A]]).astype(np.float32)

    w1bd = np.zeros((6, 128), np.float32)
    w1bd[0:3, 0:64] = W1.T
    w1bd[3:6, 64:128] = W1.T

    # reduce selectors: pass c sums rows 0:64 (even pt) into out row c and
    # rows 64:128 (odd pt) into out row 3+c
    gsel = np.zeros((3, 128, 6), np.float32)
    for c in range(3):
        gsel[c, 0:64, c] = 1.0
        gsel[c, 64:128, 3 + c] = 1.0

    bf = ml_dtypes.bfloat16
    c_ = np.ascontiguousarray
    return {
        "w1bd": c_(w1bd),
        "b1d": c_(np.concatenate([b1, b1])[:, None]),
        "w2bd": c_(bd(W2.T).astype(bf)),
        "b2d": c_(np.concatenate([b2, b2])[:, None]),
        "bBd": c_(np.stack([bd(B[c].T) for c in range(3)], axis=1).astype(bf)),
        "gsel": c_(gsel.transpose(1, 0, 2).astype(bf)),
    }


def kernel(x, W1, b1, W2, b2, W3, b3, _want_trace=False):
    x = np.asarray(x, np.float32)
    wts = _host_weights(W1, b1, W2, b2, W3)

    in_maps = []
    for ci in range(N_CORES):
        xs = x[ci * NSH:(ci + 1) * NSH]                       # (NSH, 3)
        xt6 = np.ascontiguousarray(
            xs.reshape(NSH2, 2, 3).transpose(1, 2, 0).reshape(6, NSH2))
        m = {"xt6": xt6}
        m.update(wts)
        in_maps.append(m)

    nc = _get_program()
    res = None
    for attempt in range(3):
        try:
            res = run_bass_kernel_spmd(nc, in_maps, list(range(N_CORES)),
                                       trace=_want_trace)
            break
        except Exception as e:
            # Axon-tunneled NeuronCores occasionally report a transient
            # NRT_EXEC_UNIT_UNRECOVERABLE; a retry on the same devices
            # consistently succeeds.
            if attempt == 2 or "UNRECOVERABLE" not in str(e).upper():
                raise
            import time
            time.sleep(10)
    outs = []
    for ci in range(N_CORES):
        yt6 = res.results[ci]["yt6"]                          # (6, NSH2)
        y = yt6.reshape(2, 3, NSH2).transpose(2, 0, 1).reshape(NSH, 3)
        outs.append(y)
    out = np.ascontiguousarray(np.concatenate(outs, axis=0)).astype(np.float32)
    if _want_trace:
        return out, res
    return out
